# revision 6
# baseline (speedup 1.0000x reference)
"""Instance-norm kernel for TRN2 (Bass/Tile), 8-core data-parallel.

Problem: ten (64, 3, 512, 512) f32; per-(n,c) mean and unbiased std over
(H, W); out = (x - mean) / (sqrt(var_unbiased) + 1e-8).

Sharding: batch dim N=64 split across 8 cores -> 8 batches (24 images)
per core. Each 512x512 image is viewed as an SBUF tile [128, 2048]:
  - per-partition mean/var via bn_stats/bn_aggr (DVE)
  - cross-partition reduce + broadcast in one PE matmul with a ones
    [128, 128] stationary: psum[p, :] = column sums of [m_p, E_p[x^2]]
  - scalar chain -> per-partition mean and 1/(std+eps)
  - apply (x - mean) * rstd in one DVE tensor_scalar pass
  - loads on the SP HWDGE ring, stores on the ACT HWDGE ring so the two
    streams' fixed costs overlap.
"""

from contextlib import ExitStack

import numpy as np

import concourse.bass as bass
import concourse.tile as tile
from concourse import bacc, mybir
from concourse._compat import with_exitstack
from concourse.bass_utils import run_bass_kernel_spmd

N, C, H, W = 64, 3, 512, 512
NCORES = 8
NB = N // NCORES              # batches per core
IMGS = NB * C                 # images (n,c) per core
HW = H * W                    # 262144 elements per image
P = 128                       # SBUF partitions
F = HW // P                   # 2048 free elements per partition
EPS = 1e-8
BN_FMAX = 512
NSUB = F // BN_FMAX           # bn_stats subgroups per partition

FP32 = mybir.dt.float32


G = 6                         # images per stats group
NGROUPS = IMGS // G


@with_exitstack
def _norm_body(ctx: ExitStack, tc: tile.TileContext, y: bass.AP, x: bass.AP):
    nc = tc.nc
    data = ctx.enter_context(tc.tile_pool(name="data", bufs=18))
    small = ctx.enter_context(tc.tile_pool(name="small", bufs=3))
    grp = ctx.enter_context(tc.tile_pool(name="grp", bufs=3))
    psum = ctx.enter_context(tc.tile_pool(name="psum", bufs=3, space="PSUM"))
    singles = ctx.enter_context(tc.tile_pool(name="singles", bufs=1))

    ones = singles.tile([P, P], FP32)
    nc.vector.memset(ones, 1.0)

    # sqrt(var_b * corr) turns the biased (/HW) variance into the
    # unbiased (/(HW-1)) one.
    corr = float(HW) / float(HW - 1)

    def stage_load_stats(g):
        # Raw sums for the whole group land in one [P, 2*G] tile:
        # column k = per-partition partial sum(x) of image k, column G+k =
        # partial sum(x^2). The ones-matmul then turns the partials into
        # full-image sums broadcast to every partition.
        xts = []
        mv = grp.tile([P, 2 * G], FP32, tag="mv")
        for k in range(G):
            i = g * G + k
            xt = data.tile([P, F], FP32, tag="xt")
            xts.append(xt)
            nc.sync.dma_start(out=xt[:], in_=x[i * P : (i + 1) * P, :])
            nc.vector.tensor_reduce(
                out=mv[:, k : k + 1], in_=xt[:],
                axis=mybir.AxisListType.X, op=mybir.AluOpType.add,
            )
            scr = small.tile([P, F], FP32, tag="scr")
            nc.scalar.activation(
                out=scr[:], in_=xt[:],
                func=mybir.ActivationFunctionType.Square,
                accum_out=mv[:, G + k : G + k + 1],
            )
        return xts, mv

    def stage_chain(mv):
        ps = psum.tile([P, 2 * G], FP32, tag="ps")
        nc.tensor.matmul(ps[:], ones[:], mv[:], start=True, stop=True)
        # ps[:, k] = sum(x_k), ps[:, G+k] = sum(x_k^2), on every partition.
        mean = grp.tile([P, G], FP32, tag="mean")
        nc.scalar.mul(mean[:], ps[:, 0:G], 1.0 / HW)
        mean2 = grp.tile([P, G], FP32, tag="mean2")
        nc.scalar.activation(
            mean2[:], ps[:, 0:G], func=mybir.ActivationFunctionType.Square,
            scale=1.0 / HW,
        )
        varb = grp.tile([P, G], FP32, tag="varb")
        nc.vector.scalar_tensor_tensor(
            out=varb[:], in0=ps[:, G : 2 * G], scalar=1.0 / HW, in1=mean2[:],
            op0=mybir.AluOpType.mult, op1=mybir.AluOpType.subtract,
        )
        std = grp.tile([P, G], FP32, tag="std")
        nc.scalar.activation(
            std[:], varb[:], func=mybir.ActivationFunctionType.Sqrt, scale=corr
        )
        stdp = grp.tile([P, G], FP32, tag="stdp")
        nc.vector.tensor_scalar_add(stdp[:], std[:], EPS)
        rstd = grp.tile([P, G], FP32, tag="rstd")
        nc.vector.reciprocal(rstd[:], stdp[:])
        return mean, rstd

    def stage_apply(g, xts, mean, rstd):
        for k in range(G):
            i = g * G + k
            xt = xts[k]
            nc.vector.tensor_scalar(
                out=xt[:], in0=xt[:], scalar1=mean[:, k : k + 1],
                scalar2=rstd[:, k : k + 1],
                op0=mybir.AluOpType.subtract, op1=mybir.AluOpType.mult,
            )
            nc.scalar.dma_start(out=y[i * P : (i + 1) * P, :], in_=xt[:])

    # Software pipeline: group g+1's loads/stats are emitted before group
    # g's applies so the DMA load stream never drains while the (serial)
    # stats chain of group g is in flight.
    xts, mv = stage_load_stats(0)
    for g in range(NGROUPS):
        mean, rstd = stage_chain(mv)
        if g + 1 < NGROUPS:
            nxts, nmv = stage_load_stats(g + 1)
        stage_apply(g, xts, mean, rstd)
        if g + 1 < NGROUPS:
            xts, mv = nxts, nmv


def _build():
    nc = bacc.Bacc(
        "TRN2", target_bir_lowering=False, debug=False, num_devices=NCORES
    )
    x = nc.dram_tensor("x", [IMGS * P, F], FP32, kind="ExternalInput").ap()
    y = nc.dram_tensor("y", [IMGS * P, F], FP32, kind="ExternalOutput").ap()
    with tile.TileContext(nc) as tc:
        _norm_body(tc, y, x)
    nc.finalize()
    return nc


_nc = None


def _run(ten: np.ndarray, **kw):
    global _nc
    if _nc is None:
        _nc = _build()
    shards = np.ascontiguousarray(ten, dtype=np.float32).reshape(
        NCORES, IMGS * P, F
    )
    in_maps = [{"x": shards[k]} for k in range(NCORES)]
    res = run_bass_kernel_spmd(_nc, in_maps, core_ids=list(range(NCORES)), **kw)
    out = np.stack([res.results[k]["y"] for k in range(NCORES)])
    return out.reshape(N, C, H, W), res


def kernel(**inputs: np.ndarray) -> np.ndarray:
    out, _ = _run(np.asarray(inputs["ten"]))
    return out
